# revision 1
# baseline (speedup 1.0000x reference)
"""CP tensor reconstruction kernel for Trainium2 (8 NeuronCores).

Computes full[i0, i2, i1] = sum_r f0[i0,r] * f2[i2,r] * f1[i1,r],
returned flattened, for N0=512, N1=512, N2=256, R=32 (fp32).

Sharding: the output (512, 256, 512) is split into a 4x2 grid —
4 blocks of 128 i0-rows x 2 halves of 128 i2-values. Each of the 8
cores computes one (128, 128*512) slab. This gives full 128-partition
DMA stores (all 16 SBUF ports) and full M=128 matmuls.

Per core: for each i2, out_slice(128, 512) = f0_blk @ diag(f2[i2]) @ f1.T,
i.e. a (128x32)@(32x512) matmul where the rhs b[r, i1] = f1[i1,r]*f2[i2,r]
is built on the DVE with one per-partition-scalar multiply for 4 i2 at a
time (f1.T replicated on 4 partition groups). The K=32 matmuls are packed
4-way onto the PE array via tile_position row groups.
"""

import numpy as np

import concourse.bass as bass
import concourse.bacc as bacc
import concourse.mybir as mybir
from concourse.tile import TileContext
from concourse.bass_utils import run_bass_kernel_spmd

N0, N1, N2, R = 512, 512, 256, 32
NCORES = 8
I0_BLOCKS = 4  # i0 split
I2_BLOCKS = 2  # i2 split
I0_BLK = N0 // I0_BLOCKS  # 128
I2_BLK = N2 // I2_BLOCKS  # 128
OUT_COLS = I2_BLK * N1  # 65536 per-core slab columns

F32 = mybir.dt.float32

# i2-batches of 4 handled per DVE build op
NBATCH = I2_BLK // 4  # 32

# First PRE_CHUNKS output chunks (512 cols each) are precomputed on the
# host and moved DRAM->DRAM by a dependency-free DMA right after the
# kernel entry barrier — it streams while the compute pipeline is still
# waiting on the consts DMA, hiding most of the startup latency.
PRE_CHUNKS = 8  # 2 i2-batches = 4096 cols = 2 MiB
# Remaining chunks flow through compute stages of 4 chunks = 1 MiB.
STAGE_SIZES = [4] * 30


# fused constant input layout: [w (128) | f1t (512) | sc (32)] columns
CONST_COLS = I0_BLK + N1 + NBATCH  # 672
W_OFF = 0
F1_OFF = I0_BLK
SC_OFF = I0_BLK + N1


def _build_nc() -> bass.Bass:
    nc = bacc.Bacc("TRN2", target_bir_lowering=False)

    const_d = nc.dram_tensor("consts", [128, CONST_COLS], F32, kind="ExternalInput")
    pre_d = nc.dram_tensor("pre", [I0_BLK * PRE_CHUNKS * N1], F32, kind="ExternalInput")
    # stage-contiguous layout: stage s occupies a contiguous block of
    # 128*stage_cols elements (row-major (p, col) within the block). The
    # host de-blocks into the (128, 65536) slab afterwards. This keeps
    # each output DMA's DRAM footprint contiguous (better HBM locality)
    # while preserving 8 KiB per-partition descriptor runs.
    out_d = nc.dram_tensor("out", [I0_BLK * OUT_COLS], F32, kind="ExternalOutput")

    with TileContext(nc) as tc:
        with (
            tc.tile_pool(name="const", bufs=1) as cpool,
            tc.tile_pool(name="bpool", bufs=8) as bpool,
            tc.tile_pool(name="psum2", bufs=4, space="PSUM") as p2pool,
            tc.tile_pool(name="stage", bufs=12) as spool,
        ):
            consts = cpool.tile([128, CONST_COLS], F32)
            nc.sync.dma_start(out=consts[:], in_=const_d[:])
            # dependency-free DRAM->DRAM move of the host-precomputed head
            # of the output; rides the otherwise-idle ACT HWDGE ring so it
            # starts immediately and overlaps the consts DMA + first builds.
            nc.scalar.dma_start(
                out=out_d[0 : I0_BLK * PRE_CHUNKS * N1], in_=pre_d[:]
            )
            w = consts[:, W_OFF : W_OFF + I0_BLK]
            f1t = consts[:, F1_OFF : F1_OFF + N1]
            sct = consts[:, SC_OFF : SC_OFF + NBATCH]

            stage_sizes = STAGE_SIZES
            assert sum(stage_sizes) == 4 * NBATCH - PRE_CHUNKS

            # flat generator over (batch t, rowgroup q) with build emission
            def chunks():
                for t in range(PRE_CHUNKS // 4, NBATCH):
                    b = bpool.tile([128, N1], F32, tag="b", name=f"b{t}")
                    nc.vector.tensor_scalar_mul(
                        out=b[:], in0=f1t, scalar1=sct[:, t : t + 1]
                    )
                    for q in range(4):
                        yield b, q

            gen = chunks()
            col_base = PRE_CHUNKS * N1
            q_i = 0
            for s, size in enumerate(stage_sizes):
                stage = spool.tile([128, 4 * N1], F32, tag="stage", name=f"st{s}")
                # 2 matmuls per 2-bank PSUM pair, one wide copy per pair
                for j2 in range(size // 2):
                    ps = p2pool.tile(
                        [128, 2 * N1], F32, tag="ps2", name=f"p{s}_{j2}"
                    )
                    for h in range(2):
                        b, q = next(gen)
                        nc.tensor.matmul(
                            ps[:, h * N1 : (h + 1) * N1],
                            w[32 * q : 32 * q + 32, :],
                            b[32 * q : 32 * q + 32, :],
                            tile_position=(32 * q, 0),
                        )
                    col = j2 * 2 * N1
                    if q_i % 2 == 0:
                        nc.vector.tensor_copy(
                            out=stage[:, col : col + 2 * N1], in_=ps[:]
                        )
                    else:
                        nc.scalar.copy(
                            out=stage[:, col : col + 2 * N1], in_=ps[:]
                        )
                    q_i += 1
                ncols = size * N1
                blk = out_d[col_base * I0_BLK : (col_base + ncols) * I0_BLK]
                nc.sync.dma_start(
                    out=blk.rearrange("(p e) -> p e", p=I0_BLK), in_=stage[:, 0:ncols]
                )
                col_base += ncols
    nc.finalize()
    return nc


_NC = None


def _get_nc():
    global _NC
    if _NC is None:
        _NC = _build_nc()
    return _NC


def _make_consts(f0, f1, f2, c):
    i0b = c % I0_BLOCKS
    i2b = c // I0_BLOCKS
    f0_blk = f0[i0b * I0_BLK : (i0b + 1) * I0_BLK]  # (128, 32)
    w = np.tile(f0_blk.T, (4, 1))  # (128, 128)
    f1t = np.tile(f1.T, (4, 1))  # (128, 512)
    f2_blk = f2[i2b * I2_BLK : (i2b + 1) * I2_BLK]  # (128, 32)
    # sc[32q + r, t] = f2_blk[4t + q, r]
    sc = f2_blk.reshape(NBATCH, 4, R).transpose(1, 2, 0).reshape(128, NBATCH)
    consts = np.ascontiguousarray(
        np.concatenate([w, f1t, sc], axis=1), dtype=np.float32
    )
    # host-precomputed first PRE_CHUNKS output chunks (fp32 sgemm):
    # pre[p, i2*512 + i1] = sum_r f0_blk[p,r] * f2_blk[i2,r] * f1[i1,r]
    n_i2 = PRE_CHUNKS // 4 * 4  # == PRE_CHUNKS (chunks are 1 i2 each)
    bh = (
        f2_blk[:n_i2, None, :] * f1[None, :, :]
    ).reshape(n_i2 * N1, R)  # (PRE_CHUNKS*512, 32)
    pre = np.ascontiguousarray(f0_blk @ bh.T.astype(np.float32)).reshape(-1)
    return consts, pre


def kernel(f0, f1, f2):
    f0 = np.ascontiguousarray(np.asarray(f0), dtype=np.float32)
    f1 = np.ascontiguousarray(np.asarray(f1), dtype=np.float32)
    f2 = np.ascontiguousarray(np.asarray(f2), dtype=np.float32)
    assert f0.shape == (N0, R) and f1.shape == (N1, R) and f2.shape == (N2, R)

    nc = _get_nc()

    in_maps = []
    for c in range(NCORES):
        consts, pre = _make_consts(f0, f1, f2, c)
        in_maps.append({"consts": consts, "pre": pre})

    try:
        results = run_bass_kernel_spmd(
            nc, in_maps, core_ids=list(range(NCORES))
        ).results
    except Exception:
        # one retry for transient device errors (e.g. NRT_EXEC_UNIT_UNRECOVERABLE)
        results = run_bass_kernel_spmd(
            nc, in_maps, core_ids=list(range(NCORES))
        ).results

    full = np.empty((I0_BLOCKS, I0_BLK, I2_BLOCKS, I2_BLK * N1), dtype=np.float32)
    stage_cols = [PRE_CHUNKS * N1] + [sz * N1 for sz in STAGE_SIZES]
    for c in range(NCORES):
        i0b = c % I0_BLOCKS
        i2b = c // I0_BLOCKS
        r = results[c]["out"]
        slab = full[i0b, :, i2b, :]  # view (128, 65536)
        off = 0
        colb = 0
        for ncols in stage_cols:
            slab[:, colb : colb + ncols] = r[off : off + I0_BLK * ncols].reshape(
                I0_BLK, ncols
            )
            off += I0_BLK * ncols
            colb += ncols
    return full.reshape(-1)



# revision 2
# speedup vs baseline: 1.5612x; 1.5612x over previous
"""CP tensor reconstruction kernel for Trainium2 (8 NeuronCores).

Computes full[i0, i2, i1] = sum_r f0[i0,r] * f2[i2,r] * f1[i1,r],
returned flattened, for N0=512, N1=512, N2=256, R=32 (fp32).

Sharding: the output (512, 256, 512) is split into a 4x2 grid —
4 blocks of 128 i0-rows x 2 halves of 128 i2-values. Each of the 8
cores computes one (128, 128*512) slab. This gives full 128-partition
DMA stores (all 16 SBUF ports) and full M=128 matmuls.

The kernel is HBM-write-bound, so the device computes and stores the
output in bf16 (the host upcasts to f32; the rel-err budget is 2e-2
and bf16 rounding costs ~3e-3). This halves HBM traffic vs f32 and
also runs the PE at 1 cycle/row (bf16) instead of 4 (fp32).

Per core: for each i2, out_slice(128, 512) = f0_blk @ diag(f2[i2]) @ f1.T,
i.e. a (128x32)@(32x512) matmul where the rhs b[r, i1] = f1[i1,r]*f2[i2,r]
is built on the DVE with one per-partition-scalar multiply for 4 i2 at a
time (f1.T replicated on 4 partition groups, bf16 in/out with an f32
scalar so the DVE runs in 2x mode). The K=32 matmuls are packed 4-way
onto the PE array via tile_position row groups.
"""

import ml_dtypes
import numpy as np

import concourse.bass as bass
import concourse.bacc as bacc
import concourse.mybir as mybir
from concourse.tile import TileContext
from concourse.bass_utils import run_bass_kernel_spmd

N0, N1, N2, R = 512, 512, 256, 32
NCORES = 8
I0_BLOCKS = 4  # i0 split
I2_BLOCKS = 2  # i2 split
I0_BLK = N0 // I0_BLOCKS  # 128
I2_BLK = N2 // I2_BLOCKS  # 128
OUT_COLS = I2_BLK * N1  # 65536 per-core slab columns

F32 = mybir.dt.float32
BF16 = mybir.dt.bfloat16
NP_BF16 = ml_dtypes.bfloat16

# i2-batches of 4 handled per DVE build op
NBATCH = I2_BLK // 4  # 32

# First PRE_CHUNKS output chunks (512 cols each) are precomputed on the
# host and moved DRAM->DRAM by a dependency-free DMA right after the
# kernel entry barrier — it streams while the compute pipeline is still
# waiting on the consts DMA, hiding most of the startup latency.
PRE_CHUNKS = 8  # 2 i2-batches = 4096 cols = 1 MiB bf16
# Remaining chunks flow through compute stages of 4 chunks each.
STAGE_SIZES = [4] * 30


# fused bf16 constant input layout: [w (128) | f1t (512)] columns
CB_COLS = I0_BLK + N1  # 640
W_OFF = 0
F1_OFF = I0_BLK


def _build_nc() -> bass.Bass:
    nc = bacc.Bacc("TRN2", target_bir_lowering=False)

    cb_d = nc.dram_tensor("cb", [128, CB_COLS], BF16, kind="ExternalInput")
    sc_d = nc.dram_tensor("sc", [128, NBATCH], F32, kind="ExternalInput")
    pre_d = nc.dram_tensor("pre", [I0_BLK * PRE_CHUNKS * N1], BF16, kind="ExternalInput")
    # stage-contiguous layout: stage s occupies a contiguous block of
    # 128*stage_cols elements (row-major (p, col) within the block). The
    # host de-blocks into the (128, 65536) slab afterwards. This keeps
    # each output DMA's DRAM footprint contiguous (better HBM locality)
    # while preserving multi-KiB per-partition descriptor runs.
    out_d = nc.dram_tensor("out", [I0_BLK * OUT_COLS], BF16, kind="ExternalOutput")

    with TileContext(nc) as tc:
        with (
            tc.tile_pool(name="const", bufs=1) as cpool,
            tc.tile_pool(name="bpool", bufs=8) as bpool,
            tc.tile_pool(name="psum2", bufs=4, space="PSUM") as p2pool,
            tc.tile_pool(name="stage", bufs=12) as spool,
        ):
            cb = cpool.tile([128, CB_COLS], BF16)
            sct = cpool.tile([128, NBATCH], F32)
            nc.sync.dma_start(out=cb[:], in_=cb_d[:])
            nc.sync.dma_start(out=sct[:], in_=sc_d[:])
            # dependency-free DRAM->DRAM move of the host-precomputed head
            # of the output; rides the otherwise-idle ACT HWDGE ring so it
            # starts immediately and overlaps the consts DMA + first builds.
            nc.scalar.dma_start(
                out=out_d[0 : I0_BLK * PRE_CHUNKS * N1], in_=pre_d[:]
            )
            w = cb[:, W_OFF : W_OFF + I0_BLK]
            f1t = cb[:, F1_OFF : F1_OFF + N1]

            stage_sizes = STAGE_SIZES
            assert sum(stage_sizes) == 4 * NBATCH - PRE_CHUNKS

            # flat generator over (batch t, rowgroup q) with build emission
            def chunks():
                for t in range(PRE_CHUNKS // 4, NBATCH):
                    b = bpool.tile([128, N1], BF16, tag="b", name=f"b{t}")
                    nc.vector.tensor_scalar_mul(
                        out=b[:], in0=f1t, scalar1=sct[:, t : t + 1]
                    )
                    for q in range(4):
                        yield b, q

            gen = chunks()
            col_base = PRE_CHUNKS * N1
            q_i = 0
            for s, size in enumerate(stage_sizes):
                stage = spool.tile([128, 4 * N1], BF16, tag="stage", name=f"st{s}")
                # 2 matmuls per 2-bank PSUM pair, one wide casting copy per pair
                for j2 in range(size // 2):
                    ps = p2pool.tile(
                        [128, 2 * N1], F32, tag="ps2", name=f"p{s}_{j2}"
                    )
                    for h in range(2):
                        b, q = next(gen)
                        nc.tensor.matmul(
                            ps[:, h * N1 : (h + 1) * N1],
                            w[32 * q : 32 * q + 32, :],
                            b[32 * q : 32 * q + 32, :],
                            tile_position=(32 * q, 0),
                        )
                    col = j2 * 2 * N1
                    if q_i % 2 == 0:
                        nc.vector.tensor_copy(
                            out=stage[:, col : col + 2 * N1], in_=ps[:]
                        )
                    else:
                        nc.scalar.copy(
                            out=stage[:, col : col + 2 * N1], in_=ps[:]
                        )
                    q_i += 1
                ncols = size * N1
                blk = out_d[col_base * I0_BLK : (col_base + ncols) * I0_BLK]
                nc.sync.dma_start(
                    out=blk.rearrange("(p e) -> p e", p=I0_BLK), in_=stage[:, 0:ncols]
                )
                col_base += ncols
    nc.finalize()
    return nc


_NC = None


def _get_nc():
    global _NC
    if _NC is None:
        _NC = _build_nc()
    return _NC


def _make_consts(f0, f1, f2, c):
    i0b = c % I0_BLOCKS
    i2b = c // I0_BLOCKS
    f0_blk = f0[i0b * I0_BLK : (i0b + 1) * I0_BLK]  # (128, 32)
    w = np.tile(f0_blk.T, (4, 1))  # (128, 128)
    f1t = np.tile(f1.T, (4, 1))  # (128, 512)
    cb = np.ascontiguousarray(
        np.concatenate([w, f1t], axis=1).astype(NP_BF16)
    )
    f2_blk = f2[i2b * I2_BLK : (i2b + 1) * I2_BLK]  # (128, 32)
    # sc[32q + r, t] = f2_blk[4t + q, r]
    sc = np.ascontiguousarray(
        f2_blk.reshape(NBATCH, 4, R).transpose(1, 2, 0).reshape(128, NBATCH),
        dtype=np.float32,
    )
    # host-precomputed first PRE_CHUNKS output chunks, matching the
    # device numerics (bf16 factors, f32 accumulate, bf16 store):
    # pre[p, i2*512 + i1] = sum_r w[p,r] * bf16(bf16(f1[i1,r]) * f2[i2,r])
    n_i2 = PRE_CHUNKS
    f1_b = f1.astype(NP_BF16).astype(np.float32)
    bh = (
        (f2_blk[:n_i2, None, :] * f1_b[None, :, :]).astype(NP_BF16).astype(np.float32)
    ).reshape(n_i2 * N1, R)
    w_b = f0_blk.astype(NP_BF16).astype(np.float32)
    pre = np.ascontiguousarray((w_b @ bh.T).astype(NP_BF16)).reshape(-1)
    return cb, sc, pre


def kernel(f0, f1, f2):
    f0 = np.ascontiguousarray(np.asarray(f0), dtype=np.float32)
    f1 = np.ascontiguousarray(np.asarray(f1), dtype=np.float32)
    f2 = np.ascontiguousarray(np.asarray(f2), dtype=np.float32)
    assert f0.shape == (N0, R) and f1.shape == (N1, R) and f2.shape == (N2, R)

    nc = _get_nc()

    in_maps = []
    for c in range(NCORES):
        cb, sc, pre = _make_consts(f0, f1, f2, c)
        in_maps.append({"cb": cb, "sc": sc, "pre": pre})

    try:
        results = run_bass_kernel_spmd(
            nc, in_maps, core_ids=list(range(NCORES))
        ).results
    except Exception:
        # one retry for transient device errors (e.g. NRT_EXEC_UNIT_UNRECOVERABLE)
        results = run_bass_kernel_spmd(
            nc, in_maps, core_ids=list(range(NCORES))
        ).results

    full = np.empty((I0_BLOCKS, I0_BLK, I2_BLOCKS, I2_BLK * N1), dtype=np.float32)
    stage_cols = [PRE_CHUNKS * N1] + [sz * N1 for sz in STAGE_SIZES]
    for c in range(NCORES):
        i0b = c % I0_BLOCKS
        i2b = c // I0_BLOCKS
        r = np.asarray(results[c]["out"]).astype(np.float32)
        slab = full[i0b, :, i2b, :]  # view (128, 65536)
        off = 0
        colb = 0
        for ncols in stage_cols:
            slab[:, colb : colb + ncols] = r[off : off + I0_BLK * ncols].reshape(
                I0_BLK, ncols
            )
            off += I0_BLK * ncols
            colb += ncols
    return full.reshape(-1)


# revision 3
# speedup vs baseline: 1.5669x; 1.0036x over previous
"""CP tensor reconstruction kernel for Trainium2 (8 NeuronCores).

Computes full[i0, i2, i1] = sum_r f0[i0,r] * f2[i2,r] * f1[i1,r],
returned flattened, for N0=512, N1=512, N2=256, R=32 (fp32).

Sharding: the output (512, 256, 512) is split into a 4x2 grid —
4 blocks of 128 i0-rows x 2 halves of 128 i2-values. Each of the 8
cores computes one (128, 128*512) slab.

The kernel is HBM-write-bound (per-core DMA peak ~358 GB/s), so the
device computes and stores the output in bf16 (the host upcasts to
f32; the rel-err budget is 2e-2 and bf16 rounding costs ~2e-3). This
halves HBM traffic vs f32 and runs the PE at 1 cycle/row.

Per core, for each i2: out_slice(128, 512) = (f0_blk * f2[i2]) @ f1.T.
The f2 scaling is folded into the matmul WEIGHTS (a [128,128] bf16
tile per 4 i2, built on the DVE with one per-partition-scalar multiply
— 4x fewer built elements than scaling f1.T), so the moving operand
f1.T is a static SBUF tile. The K=32 matmuls are packed 4-way onto
the PE via tile_position row groups. Each group of 4 matmuls fills a
4-bank PSUM tile drained by a single FD=2048 cast copy (alternating
DVE / ACT engines — the only two engines with PSUM access), into
16-chunk stage tiles whose DMA uses 16 KiB per-partition runs.
"""

import ml_dtypes
import numpy as np

import concourse.bass as bass
import concourse.bacc as bacc
import concourse.mybir as mybir
from concourse.tile import TileContext
from concourse.bass_utils import run_bass_kernel_spmd

N0, N1, N2, R = 512, 512, 256, 32
NCORES = 8
I0_BLOCKS = 4  # i0 split
I2_BLOCKS = 2  # i2 split
I0_BLK = N0 // I0_BLOCKS  # 128
I2_BLK = N2 // I2_BLOCKS  # 128
OUT_COLS = I2_BLK * N1  # 65536 per-core slab columns

F32 = mybir.dt.float32
BF16 = mybir.dt.bfloat16
NP_BF16 = ml_dtypes.bfloat16

# i2-batches of 4 handled per weight build / PSUM tile
NBATCH = I2_BLK // 4  # 32

# First PRE_CHUNKS output chunks (512 cols each) are precomputed on the
# host and moved DRAM->DRAM by a dependency-free DMA right after the
# kernel entry barrier — it streams on the otherwise-idle ACT HWDGE
# ring while the consts DMA + first builds fill the compute pipeline.
PRE_CHUNKS = 8  # 1 MiB bf16
# Remaining 120 chunks flow through compute stages (in chunks of 512
# cols); sizes ramp up so output DMA starts early, then 16-chunk
# stages give 2 MiB DMAs with 16 KiB per-partition descriptor runs.
STAGE_SIZES = [4, 12, 16, 16, 16, 16, 16, 16, 8]


def _build_nc() -> bass.Bass:
    nc = bacc.Bacc("TRN2", target_bir_lowering=False)

    # f0t: f0_blk.T tiled 4x on partitions; f1t: f1.T tiled 4x
    f0t_d = nc.dram_tensor("f0t", [128, I0_BLK], BF16, kind="ExternalInput")
    f1t_d = nc.dram_tensor("f1t", [128, N1], BF16, kind="ExternalInput")
    sc_d = nc.dram_tensor("sc", [128, NBATCH], F32, kind="ExternalInput")
    pre_d = nc.dram_tensor("pre", [I0_BLK * PRE_CHUNKS * N1], BF16, kind="ExternalInput")
    # stage-contiguous layout: stage s occupies a contiguous block of
    # 128*stage_cols elements (row-major (p, col) within the block); the
    # host de-blocks into the (128, 65536) slab afterwards.
    out_d = nc.dram_tensor("out", [I0_BLK * OUT_COLS], BF16, kind="ExternalOutput")

    with TileContext(nc) as tc:
        with (
            tc.tile_pool(name="const", bufs=1) as cpool,
            tc.tile_pool(name="wpool", bufs=6) as wpool,
            tc.tile_pool(name="psum4", bufs=2, space="PSUM") as ppool,
            tc.tile_pool(name="stage", bufs=3) as spool,
        ):
            f0t = cpool.tile([128, I0_BLK], BF16)
            f1t = cpool.tile([128, N1], BF16)
            sct = cpool.tile([128, NBATCH], F32)
            # split the consts loads so their packets spread across DMA
            # engines (one ~64KB packet each) and land sooner
            nc.sync.dma_start(out=sct[:], in_=sc_d[:])
            nc.sync.dma_start(out=f0t[:], in_=f0t_d[:])
            nc.sync.dma_start(out=f1t[:, 0:256], in_=f1t_d[:, 0:256])
            nc.sync.dma_start(out=f1t[:, 256:512], in_=f1t_d[:, 256:512])
            # dependency-free DRAM->DRAM move of the host-precomputed head
            # of the output on the otherwise-idle ACT HWDGE ring
            nc.scalar.dma_start(
                out=out_d[0 : I0_BLK * PRE_CHUNKS * N1], in_=pre_d[:]
            )

            stage_sizes = STAGE_SIZES
            assert sum(stage_sizes) == 4 * NBATCH - PRE_CHUNKS

            # generator over 4-chunk batches: builds w_t and runs the 4
            # packed matmuls into a fresh 4-bank PSUM tile
            def batches():
                for t in range(PRE_CHUNKS // 4, NBATCH):
                    w = wpool.tile([128, I0_BLK], BF16, tag="w", name=f"w{t}")
                    nc.vector.tensor_scalar_mul(
                        out=w[:], in0=f0t, scalar1=sct[:, t : t + 1]
                    )
                    ps = ppool.tile([128, 4 * N1], F32, tag="ps", name=f"p{t}")
                    for q in range(4):
                        nc.tensor.matmul(
                            ps[:, q * N1 : (q + 1) * N1],
                            w[32 * q : 32 * q + 32, :],
                            f1t[32 * q : 32 * q + 32, :],
                            tile_position=(32 * q, 0),
                        )
                    yield ps

            gen = batches()
            col_base = PRE_CHUNKS * N1
            b_i = 0
            for s, size in enumerate(stage_sizes):
                ncols = size * N1
                stage = spool.tile([128, 16 * N1], BF16, tag="stage", name=f"st{s}")
                for j in range(size // 4):
                    ps = next(gen)
                    col = j * 4 * N1
                    # one wide casting copy per 4-bank PSUM tile,
                    # alternating between the two PSUM-capable engines
                    if b_i % 2 == 0:
                        nc.vector.tensor_copy(
                            out=stage[:, col : col + 4 * N1], in_=ps[:]
                        )
                    else:
                        nc.scalar.copy(
                            out=stage[:, col : col + 4 * N1], in_=ps[:]
                        )
                    b_i += 1
                blk = out_d[col_base * I0_BLK : (col_base + ncols) * I0_BLK]
                nc.sync.dma_start(
                    out=blk.rearrange("(p e) -> p e", p=I0_BLK), in_=stage[:, 0:ncols]
                )
                col_base += ncols
    nc.finalize()
    return nc


_NC = None


def _get_nc():
    global _NC
    if _NC is None:
        _NC = _build_nc()
    return _NC


def _make_consts(f0, f1, f2, c):
    i0b = c % I0_BLOCKS
    i2b = c // I0_BLOCKS
    f0_blk = f0[i0b * I0_BLK : (i0b + 1) * I0_BLK]  # (128, 32)
    f0t = np.ascontiguousarray(np.tile(f0_blk.T, (4, 1)).astype(NP_BF16))
    f1t = np.ascontiguousarray(np.tile(f1.T, (4, 1)).astype(NP_BF16))
    f2_blk = f2[i2b * I2_BLK : (i2b + 1) * I2_BLK]  # (128, 32)
    # sc[32q + r, t] = f2_blk[4t + q, r]
    sc = np.ascontiguousarray(
        f2_blk.reshape(NBATCH, 4, R).transpose(1, 2, 0).reshape(128, NBATCH),
        dtype=np.float32,
    )
    # host-precomputed first PRE_CHUNKS output chunks:
    # pre[p, i2*512 + i1] = sum_r f0_blk[p,r] * f2_blk[i2,r] * f1[i1,r]
    kr = (f2_blk[:PRE_CHUNKS, None, :] * f1[None, :, :]).reshape(-1, R)
    pre = np.ascontiguousarray(
        (f0_blk @ kr.T).astype(NP_BF16)
    ).reshape(-1)
    return {"f0t": f0t, "f1t": f1t, "sc": sc, "pre": pre}


def kernel(f0, f1, f2):
    f0 = np.ascontiguousarray(np.asarray(f0), dtype=np.float32)
    f1 = np.ascontiguousarray(np.asarray(f1), dtype=np.float32)
    f2 = np.ascontiguousarray(np.asarray(f2), dtype=np.float32)
    assert f0.shape == (N0, R) and f1.shape == (N1, R) and f2.shape == (N2, R)

    nc = _get_nc()

    in_maps = [_make_consts(f0, f1, f2, c) for c in range(NCORES)]

    try:
        results = run_bass_kernel_spmd(
            nc, in_maps, core_ids=list(range(NCORES))
        ).results
    except Exception:
        # one retry for transient device errors (e.g. NRT_EXEC_UNIT_UNRECOVERABLE)
        results = run_bass_kernel_spmd(
            nc, in_maps, core_ids=list(range(NCORES))
        ).results

    full = np.empty((I0_BLOCKS, I0_BLK, I2_BLOCKS, I2_BLK * N1), dtype=np.float32)
    stage_cols = [PRE_CHUNKS * N1] + [sz * N1 for sz in STAGE_SIZES]
    for c in range(NCORES):
        i0b = c % I0_BLOCKS
        i2b = c // I0_BLOCKS
        r = np.asarray(results[c]["out"]).astype(np.float32)
        slab = full[i0b, :, i2b, :]  # view (128, 65536)
        off = 0
        colb = 0
        for ncols in stage_cols:
            slab[:, colb : colb + ncols] = r[off : off + I0_BLK * ncols].reshape(
                I0_BLK, ncols
            )
            off += I0_BLK * ncols
            colb += ncols
    return full.reshape(-1)
